# revision 30
# baseline (speedup 1.0000x reference)
"""Trainium2 Bass kernel for nn_MultiHeadAttention (B=8, S=1024, D=1024, h=16).

Sharding: pure data-parallel over batch — each of the 8 NeuronCores computes
the full MHA for one batch element. No collectives.

Per-core design (bf16 matmul operands, fp32 PSUM accumulation). The softmax
exp chain on ScalarE (16 ACTIVATEs x ~1.1us per head pair = ~16.3us/pair) is
the steady-state floor, so the whole kernel is built as a software pipeline
over head pairs that keeps ScalarE saturated:

  iteration d: scores(d) matmuls (throttled to exp rate by the psum ring)
  with TWO rider streams interleaved per key block — PV(d-1) matmul groups
  and the q/k projection chunks for pair d+1. PE, ScalarE and VectorE all
  stay ~90% busy at a ~17-19us pair cadence.

PSUM is split into three pools so the long-lived projection accumulators
never sit in the scores ring's reuse path:
  "big"  2 x [128,2,512] (4 banks): scores spA/spB, v_proj accs, transposes
  "half" 2 x [128,512]   (2 banks): q/k projection accumulators
  "vph"  2 x [128,512]   (2 banks): PV accumulators (65 rows: 64 V + ones)

Softmax denominators ride as a 65th ones-column inside the PV stationary;
the division runs off a SBUF staging copy via a DRAM-spread reciprocal
broadcast on GpSimd DMAs (off the PE/ACT critical path).

o_proj opens db0-6 accumulation chains before the last pair's division
lands (ot[7] is the only missing input), then closes them with the db7
contribution + relu as it arrives.
"""
import itertools
import os
from contextlib import ExitStack

import numpy as np

import concourse.bass as bass
import concourse.tile as tile
from concourse import mybir
from concourse.bass_utils import run_bass_kernel_spmd
from concourse.masks import make_identity

f32 = mybir.dt.float32
bf16 = mybir.dt.bfloat16
AF = mybir.ActivationFunctionType
ALU = mybir.AluOpType

S = 1024
D = 1024
H = 16
DK = 64
P = 128
NB = D // P  # 8 blocks
QC = 512
N_CORES = 8


def _split_wide_waits(nc, max_waits=1):
    """This walrus build rejects instructions carrying more than one
    semaphore wait; move excess waits onto NoOp carriers inserted before
    the offending instruction on the same engine."""
    for bb in nc.m.functions[0].blocks:
        idx = 0
        while idx < len(bb.instructions):
            ins = bb.instructions[idx]
            si = ins.sync_info
            if si is not None and si.on_wait and len(si.on_wait) > max_waits:
                waits = list(si.on_wait)
                rest, keep = waits[:-max_waits], waits[-max_waits:]
                for j in range(0, len(rest), max_waits):
                    nop = mybir.InstNoOp(
                        name=f"I-waitsplit-{nc.next_id()}",
                        engine=ins.engine,
                        ins=[],
                        outs=[],
                    )
                    nop.sync_info = mybir.SyncInfo(
                        on_wait=rest[j : j + max_waits], on_update=[]
                    )
                    nc.register_instruction(nop)
                    bb.instructions.insert(idx, nop)
                    idx += 1
                ins.sync_info = mybir.SyncInfo(
                    on_wait=keep, on_update=list(si.on_update)
                )
            idx += 1


def _build_nc(with_bv: bool, with_bo: bool):
    nc = bass.Bass("TRN2", target_bir_lowering=False, debug=False, num_devices=1)

    Qd = nc.dram_tensor("Q", [S, D], f32, kind="ExternalInput").ap()
    Kd = nc.dram_tensor("K", [S, D], f32, kind="ExternalInput").ap()
    Vd = nc.dram_tensor("V", [S, D], f32, kind="ExternalInput").ap()
    WQd = nc.dram_tensor("WQ", [D, D], f32, kind="ExternalInput").ap()
    WKd = nc.dram_tensor("WK", [D, D], f32, kind="ExternalInput").ap()
    WVd = nc.dram_tensor("WV", [D, D], f32, kind="ExternalInput").ap()
    WOd = nc.dram_tensor("WO", [D, D], f32, kind="ExternalInput").ap()
    bQd = nc.dram_tensor("bQ", [D], f32, kind="ExternalInput").ap()
    bKd = nc.dram_tensor("bK", [D], f32, kind="ExternalInput").ap()
    bVd = nc.dram_tensor("bV", [D], f32, kind="ExternalInput").ap()
    bOd = nc.dram_tensor("bO", [D], f32, kind="ExternalInput").ap()
    outd = nc.dram_tensor("out", [S, D], f32, kind="ExternalOutput").ap()

    with tile.TileContext(nc) as tc, ExitStack() as ctx:
        sb = ctx.enter_context(tc.tile_pool(name="sb", bufs=1))
        ps = ctx.enter_context(tc.tile_pool(name="ps", bufs=1, space="PSUM"))
        dramp = ctx.enter_context(tc.tile_pool(name="dram", bufs=1, space="DRAM"))

        # ---- constants -------------------------------------------------
        ident = sb.tile([P, P], f32, tag="ident", name="ident")
        make_identity(nc, ident)
        identb = sb.tile([P, P], bf16, tag="identb", name="identb")
        nc.vector.tensor_copy(identb, ident)
        bqk = sb.tile([P, 2 * NB], f32, tag="bqk", name="bqk")
        nc.sync.dma_start(bqk[:, 0:NB], bQd.rearrange("(db p) -> p db", p=P))
        nc.sync.dma_start(bqk[:, NB : 2 * NB], bKd.rearrange("(db p) -> p db", p=P))
        if with_bv:
            bvb = sb.tile([P, D], f32, tag="bvb", name="bvb")
            nc.sync.dma_start(bvb, bVd[None, :].broadcast_to([P, D]))
        if with_bo:
            bob = sb.tile([P, D], f32, tag="bob", name="bob")
            nc.sync.dma_start(bob, bOd[None, :].broadcast_to([P, D]))

        def big_tile(name):
            return ps.tile([P, 2, QC], f32, tag="big", bufs=2, name=name)

        def half_tile(name, tag="half"):
            return ps.tile([P, QC], f32, tag=tag, bufs=2, name=name)

        def wload(Wd, kb, chunk, tag="wper", bufs=16):
            """Stream a [128, 512] f32 weight strip and cast to bf16.
            wv (v_proj) and wo (o_proj) share one 16-slot ring: wo's allocs
            reuse wv's slots long after v_proj consumed them."""
            wstage = sb.tile([P, QC], f32, tag="wstage", bufs=3, name="wstage")
            nc.sync.dma_start(
                wstage, Wd[kb * P : (kb + 1) * P, chunk * QC : (chunk + 1) * QC]
            )
            wb = sb.tile([P, QC], bf16, tag=tag, bufs=bufs, name=tag)
            nc.vector.tensor_copy(wb, wstage)
            return wb

        def wload_d(Wd, d):
            """The full [1024, 128] weight column block for projection d as
            one DMA (3D access pattern) + one cast — keeps the Sync queue
            and VectorE free of per-strip trigger/cast storms. Returns a
            [128, kb, 128] bf16 tile; stationary for block kb is [:, kb, :]."""
            wds = sb.tile([P, NB, P], f32, tag="wds", bufs=2, name="wds")
            nc.sync.dma_start(
                wds,
                Wd[:, d * P : (d + 1) * P].rearrange("(kb p) c -> p kb c", p=P),
            )
            wd = sb.tile([P, NB, P], bf16, tag="wd", bufs=4, name="wd")
            nc.vector.tensor_copy(wd, wds)
            return wd

        def load_transpose(Xd, slot0):
            """HBM row-major -> feature-major bf16 tiles xt[db] (128 x 1024).
            All three tensors share one 16-slot ring; V uses slots 0-7 first,
            Q takes 8-15, K reuses 0-7 once v_proj has consumed them.
            Transpose psum rotates across all three pools (6-deep ring)."""
            xt = [
                sb.tile([P, S], bf16, tag="xt", bufs=16, name=f"xt{slot0 + i}")
                for i in range(NB)
            ]
            for sblk in range(NB):
                xn = sb.tile([P, D], f32, tag="xn", bufs=2, name="xn")
                nc.sync.dma_start(xn, Xd[sblk * P : (sblk + 1) * P, :])
                xnb = sb.tile([P, D], bf16, tag="xnb", bufs=2, name="xnb")
                nc.vector.tensor_copy(xnb, xn)
                for db in range(NB):
                    r = (sblk * NB + db) % 3
                    if r == 0:
                        tp = big_tile("tp")[:, 0, :]
                    elif r == 1:
                        tp = half_tile("tph")
                    else:
                        tp = half_tile("tpv", tag="vph")
                    tpb = tp[:, 0:P].bitcast(bf16)[:, 0:P]
                    nc.tensor.transpose(
                        tpb, xnb[:, db * P : (db + 1) * P], identb
                    )
                    dst = xt[db][:, sblk * P : (sblk + 1) * P]
                    if (sblk + db) % 2 == 0:
                        nc.vector.tensor_copy(dst, tpb)
                    else:
                        nc.scalar.activation(dst, tpb, AF.Copy)
            return xt

        # ---- preps: V first (v_proj consumes vt before k_prep reuses the
        # ring slots — k_prep MUST be emitted after v_proj or the PE queue
        # deadlocks) -----------------------------------------------------
        with nc.named_scope("v_prep"):
            vt = load_transpose(Vd, 0)
        with nc.named_scope("q_prep"):
            qt = load_transpose(Qd, 8)

        # ---- V projection -> vaug (row-major, head-major, 65th=ones) ----
        with nc.named_scope("v_proj"):
            vaug = [
                sb.tile([P, H * 65], bf16, tag="vaug", bufs=NB, name=f"vaug{i}")
                for i in range(NB)
            ]
            for sblk in range(NB):
                nc.vector.memset(
                    vaug[sblk].rearrange("p (h c) -> p h c", c=65)[:, :, 64:65],
                    1.0,
                )
            wv = [[wload(WVd, kb, c) for c in range(2)] for kb in range(NB)]
            for sblk in range(NB):
                acc = big_tile("vacc")
                for kb in range(NB):
                    for c in range(2):
                        nc.tensor.matmul(
                            acc[:, c, :],
                            vt[kb][:, sblk * P : (sblk + 1) * P],
                            wv[kb][c],
                            start=(kb == 0),
                            stop=(kb == NB - 1),
                        )
                for c in range(2):
                    if with_bv:
                        nc.vector.tensor_add(
                            acc[:, c, :], acc[:, c, :], bvb[:, c * QC : (c + 1) * QC]
                        )
                    dst = vaug[sblk].rearrange("p (h c) -> p h c", c=65)[
                        :, c * 8 : (c + 1) * 8, 0:64
                    ]
                    nc.scalar.activation(
                        dst,
                        acc[:, c, :].rearrange("p (h c) -> p h c", c=64),
                        AF.Relu,
                    )

        # k_prep reuses V's xt ring slots — must come after v_proj
        with nc.named_scope("k_prep"):
            kt = load_transpose(Kd, 0)

        # ---- per-pair pipeline pieces -----------------------------------
        def gen_projd(xt, wcol, d, bias_base, tag, out):
            """Projection output block relu(W[:, dblk].T @ X^T + b) as a
            generator yielding after each 4-matmul chunk (rider stream).
            Appends the evicted feature-major bf16 ring tile to `out`."""
            acc0 = half_tile("pacc0")
            acc1 = half_tile("pacc1")
            for g in range(4):
                for kb in (2 * g, 2 * g + 1):
                    wt = wcol[:, kb, :]
                    first, last = kb == 0, kb == NB - 1
                    nc.tensor.matmul(
                        acc0, wt, xt[kb][:, 0:QC], start=first, stop=last
                    )
                    nc.tensor.matmul(
                        acc1, wt, xt[kb][:, QC:S], start=first, stop=last
                    )
                yield
            xpt = sb.tile([P, S], bf16, tag=tag, bufs=2, name=tag)
            for c, acc in ((0, acc0), (1, acc1)):
                nc.vector.tensor_scalar(
                    out=xpt[:, c * QC : (c + 1) * QC],
                    in0=acc,
                    scalar1=bqk[:, bias_base + d : bias_base + d + 1],
                    scalar2=0.0,
                    op0=ALU.add,
                    op1=ALU.max,
                )
            out.append(xpt)

        def emit_scores_unit(d, qpt, kpt, riders):
            """Scores + exp for head pair d (even head on PE rows 0-63, odd
            on 64-127, concurrent in the array). After each key block the
            rider generators (PV of pair d-1, projections for pair d+1) get
            one step each, so the PE stays busy while the scores matmuls
            throttle on the exp-gated psum ring."""
            ptA = sb.tile([P, NB, 2, QC], bf16, tag="pt", bufs=4, name="ptA")
            ptB = sb.tile([P, NB, 2, QC], bf16, tag="pt", bufs=4, name="ptB")
            for kb in range(NB):
                ksl = slice(kb * P, (kb + 1) * P)
                spA = big_tile("spA")
                spB = big_tile("spB")
                for qc in range(2):
                    qsl = slice(qc * QC, (qc + 1) * QC)
                    nc.tensor.matmul(
                        spA[:, qc, :], kpt[0:DK, ksl], qpt[0:DK, qsl],
                        start=True, stop=True,
                    )
                for qc in range(2):
                    qsl = slice(qc * QC, (qc + 1) * QC)
                    nc.tensor.matmul(
                        spB[:, qc, :], kpt[DK:P, ksl], qpt[DK:P, qsl],
                        start=True, stop=True,
                    )
                nc.scalar.activation(ptA[:, kb, :, :], spA, AF.Exp, scale=0.03125)
                nc.scalar.activation(ptB[:, kb, :, :], spB, AF.Exp, scale=0.03125)
                for rd in riders:
                    if rd is not None:
                        next(rd, None)
            return ptA, ptB

        ot = [
            sb.tile([P, S], bf16, tag="ot", bufs=NB, name=f"ot{i}")
            for i in range(NB)
        ]

        def emit_pv_tail(h, vp):
            """Softmax division: reciprocal of the denominator row on a
            DRAM-spread layout + DMA broadcast + multiply (all off PE/ACT)."""
            dbq, off = h // 2, (h % 2) * DK
            for qc in range(2):
                qsl = slice(qc * QC, (qc + 1) * QC)
                stage = sb.tile([65, QC], f32, tag="stage", bufs=2, name="stage")
                nc.vector.tensor_copy(stage, vp[qc][0:65, :])
                scr = dramp.tile([1, QC], f32, tag="scr", bufs=6, name="scr")
                nc.gpsimd.dma_start(scr, stage[64:65, :])
                rcp = sb.tile([DK, NB], f32, tag="rcp", bufs=3, name="rcp")
                nc.gpsimd.dma_start(
                    rcp, scr.rearrange("o (a b) -> a (o b)", a=DK)
                )
                nc.vector.reciprocal(rcp, rcp)
                scr2 = dramp.tile([1, QC], f32, tag="scr2", bufs=6, name="scr2")
                nc.gpsimd.dma_start(
                    scr2.rearrange("o (a b) -> a (o b)", a=DK), rcp
                )
                bc = sb.tile([DK, QC], f32, tag="bc", bufs=2, name="bc")
                nc.gpsimd.dma_start(bc, scr2.broadcast_to([DK, QC]))
                if off == 0:
                    nc.vector.tensor_mul(ot[dbq][0:DK, qsl], stage[0:DK, :], bc)
                else:
                    tmp = sb.tile([DK, QC], bf16, tag="tmp", bufs=1, name="tmp")
                    nc.vector.tensor_mul(tmp, stage[0:DK, :], bc)
                    nc.gpsimd.dma_start(ot[dbq][DK:P, qsl], tmp)

        def gen_pv_pair(d, ptA, ptB):
            """PV + division for head pair (2d, 2d+1), both q-chunks, yielded
            in 8 groups of 4 matmuls for interleaving with other PE work."""
            for hl, ptX in ((0, ptA), (1, ptB)):
                h = 2 * d + hl
                vp = [half_tile(f"vp{h}_{qc}", tag="vph") for qc in range(2)]
                for g in range(4):
                    for kb in (2 * g, 2 * g + 1):
                        for qc in range(2):
                            nc.tensor.matmul(
                                vp[qc][0:65, :],
                                vaug[kb][:, h * 65 : (h + 1) * 65],
                                ptX[:, kb, qc, :],
                                start=(kb == 0),
                                stop=(kb == NB - 1),
                            )
                    yield
                emit_pv_tail(h, vp)

        # ---- pipeline: preamble proj(0), then fused iterations ----------
        with nc.named_scope("proj0"):
            wnext = [wload_d(WKd, 0), wload_d(WQd, 0)]
            cur = []
            for _ in itertools.chain(
                gen_projd(kt, wnext[0], 0, NB, "kpt", cur),
                gen_projd(qt, wnext[1], 0, 0, "qpt", cur),
            ):
                pass
            kpt_c, qpt_c = cur

        wo = []
        pend = None
        for d in range(NB):
            with nc.named_scope(f"it{d}"):
                if d + 1 < NB:
                    wnext = [wload_d(WKd, d + 1), wload_d(WQd, d + 1)]
                    nxt = []
                    g_proj = itertools.chain(
                        gen_projd(kt, wnext[0], d + 1, NB, "kpt", nxt),
                        gen_projd(qt, wnext[1], d + 1, 0, "qpt", nxt),
                    )
                else:
                    nxt, g_proj = None, None
                g_pv = gen_pv_pair(d - 1, *pend) if pend is not None else None
                if g_pv is not None:
                    next(g_pv, None)  # prime one group for iteration start
                pend = emit_scores_unit(d, qpt_c, kpt_c, [g_pv, g_proj])
                for g in (g_pv, g_proj):
                    if g is not None:
                        for _ in g:
                            pass
                if nxt is not None:
                    kpt_c, qpt_c = nxt
                if 1 <= d < 4:
                    # o_proj weight loads ride iterations 1-3 (the wper ring
                    # slots are free once v_proj is done; loading here keeps
                    # the Sync/DVE queues calm in the later iterations)
                    wo.append([wload(WOd, 2 * (d - 1), c) for c in range(2)])
                    wo.append(
                        [wload(WOd, 2 * (d - 1) + 1, c) for c in range(2)]
                    )
                    if d == 3:
                        wo.append([wload(WOd, 6, c) for c in range(2)])
                        wo.append([wload(WOd, 7, c) for c in range(2)])

        # ---- tail: o_proj chains (db 0-6 open early; db7 lands last) ----
        def oproj_open(sblk, accs, rider=None):
            for db in range(NB - 1):
                for c in range(2):
                    nc.tensor.matmul(
                        accs[c],
                        ot[db][:, sblk * P : (sblk + 1) * P],
                        wo[db][c],
                        start=(db == 0),
                        stop=False,
                    )
                if rider is not None and db % 2 == 1:
                    next(rider, None)
            return accs

        def oproj_close(sblk, accs):
            for c in range(2):
                nc.tensor.matmul(
                    accs[c],
                    ot[NB - 1][:, sblk * P : (sblk + 1) * P],
                    wo[NB - 1][c],
                    start=False,
                    stop=True,
                )
            for c in range(2):
                if with_bo:
                    nc.vector.tensor_add(
                        accs[c], accs[c], bob[:, c * QC : (c + 1) * QC]
                    )
                o = sb.tile([P, QC], f32, tag="obuf", bufs=4, name="obuf")
                nc.scalar.activation(o, accs[c], AF.Relu)
                # gpsimd (SWDGE) queue — idle at the tail, unlike Sync
                nc.gpsimd.dma_start(
                    outd[sblk * P : (sblk + 1) * P, c * QC : (c + 1) * QC], o
                )

        def chain_accs(sblk):
            """Chain accumulators: sblk 0,1,4,5 use the two big-pool tiles;
            2,3,6,7 pair a half tile with a vph tile (both free at the tail)."""
            if sblk % 4 < 2:
                t = big_tile(f"oacc{sblk}")
                return [t[:, 0, :], t[:, 1, :]]
            return [half_tile(f"oh{sblk}"), half_tile(f"ov{sblk}", tag="vph")]

        with nc.named_scope("o_proj"):
            gpv = gen_pv_pair(NB - 1, *pend)
            next(gpv, None)
            chains = {}
            # big-pool chains ride the last PV pair; the split (half+vph)
            # chains may only allocate after gpv fully drains, else their
            # matmuls wait on vph stage-copies that sit later in the PE FIFO
            for sblk in range(2):
                chains[sblk] = oproj_open(sblk, chain_accs(sblk), rider=gpv)
            for _ in gpv:
                pass
            for sblk in range(2, 4):
                chains[sblk] = oproj_open(sblk, chain_accs(sblk))
            for sblk in range(4, NB):
                oproj_close(sblk - 4, chains.pop(sblk - 4))
                chains[sblk] = oproj_open(sblk, chain_accs(sblk))
            for sblk in range(NB - 4, NB):
                oproj_close(sblk, chains.pop(sblk))

    _split_wide_waits(nc)
    return nc


_NC_CACHE = {}


def kernel(Q, K, V, WQ, bQ, WK, bK, WV, bV, WO, bO, h):
    Q, K, V = (np.ascontiguousarray(np.asarray(x, np.float32)) for x in (Q, K, V))
    WQ, WK, WV, WO = (
        np.ascontiguousarray(np.asarray(x, np.float32)) for x in (WQ, WK, WV, WO)
    )
    bQ, bK, bV, bO = (
        np.ascontiguousarray(np.asarray(x, np.float32)) for x in (bQ, bK, bV, bO)
    )
    h = int(np.asarray(h))
    assert h == H, f"kernel specialized for h=16, got {h}"
    B = Q.shape[0]
    assert Q.shape == (B, S, D) and B == N_CORES

    key = (bool(np.any(bV)), bool(np.any(bO)))
    if key not in _NC_CACHE:
        _NC_CACHE[key] = _build_nc(*key)
    nc = _NC_CACHE[key]

    in_maps = [
        {
            "Q": Q[b], "K": K[b], "V": V[b],
            "WQ": WQ, "WK": WK, "WV": WV, "WO": WO,
            "bQ": bQ, "bK": bK, "bV": bV, "bO": bO,
        }
        for b in range(B)
    ]
    trace = os.environ.get("BASS_MHA_TRACE") == "1"
    res = run_bass_kernel_spmd(
        nc, in_maps, core_ids=list(range(N_CORES)), trace=trace
    )
    if trace:
        kernel.last_results = res
    return np.stack([res.results[b]["out"] for b in range(B)], axis=0)


# revision 31
# speedup vs baseline: 1.0536x; 1.0536x over previous
"""Trainium2 Bass kernel for nn_MultiHeadAttention (B=8, S=1024, D=1024, h=16).

Sharding: pure data-parallel over batch — each of the 8 NeuronCores computes
the full MHA for one batch element. No collectives.

Per-core design (bf16 matmul operands, fp32 PSUM accumulation). The softmax
exp chain on ScalarE (16 ACTIVATEs x ~1.1us per head pair = ~16.3us/pair) is
the steady-state floor, so the whole kernel is built as a software pipeline
over head pairs that keeps ScalarE saturated:

  iteration d: scores(d) matmuls (throttled to exp rate by the psum ring)
  with TWO rider streams interleaved per key block — PV(d-1) matmul groups
  and the q/k projection chunks for pair d+1. PE, ScalarE and VectorE all
  stay ~90% busy at a ~17-19us pair cadence.

PSUM is split into three pools so the long-lived projection accumulators
never sit in the scores ring's reuse path:
  "big"  2 x [128,2,512] (4 banks): scores spA/spB, v_proj accs, transposes
  "half" 2 x [128,512]   (2 banks): q/k projection accumulators
  "vph"  2 x [128,512]   (2 banks): PV accumulators (65 rows: 64 V + ones)

Softmax denominators ride as a 65th ones-column inside the PV stationary;
the division runs off a SBUF staging copy via a DRAM-spread reciprocal
broadcast on GpSimd DMAs (off the PE/ACT critical path).

o_proj opens db0-6 accumulation chains before the last pair's division
lands (ot[7] is the only missing input), then closes them with the db7
contribution + relu as it arrives.
"""
import itertools
import os
from contextlib import ExitStack

import numpy as np

import concourse.bass as bass
import concourse.tile as tile
from concourse import mybir
from concourse.bass_utils import run_bass_kernel_spmd
from concourse.masks import make_identity

f32 = mybir.dt.float32
bf16 = mybir.dt.bfloat16
AF = mybir.ActivationFunctionType
ALU = mybir.AluOpType

S = 1024
D = 1024
H = 16
DK = 64
P = 128
NB = D // P  # 8 blocks
QC = 512
N_CORES = 8


def _split_wide_waits(nc, max_waits=1):
    """This walrus build rejects instructions carrying more than one
    semaphore wait; move excess waits onto NoOp carriers inserted before
    the offending instruction on the same engine."""
    for bb in nc.m.functions[0].blocks:
        idx = 0
        while idx < len(bb.instructions):
            ins = bb.instructions[idx]
            si = ins.sync_info
            if si is not None and si.on_wait and len(si.on_wait) > max_waits:
                waits = list(si.on_wait)
                rest, keep = waits[:-max_waits], waits[-max_waits:]
                for j in range(0, len(rest), max_waits):
                    nop = mybir.InstNoOp(
                        name=f"I-waitsplit-{nc.next_id()}",
                        engine=ins.engine,
                        ins=[],
                        outs=[],
                    )
                    nop.sync_info = mybir.SyncInfo(
                        on_wait=rest[j : j + max_waits], on_update=[]
                    )
                    nc.register_instruction(nop)
                    bb.instructions.insert(idx, nop)
                    idx += 1
                ins.sync_info = mybir.SyncInfo(
                    on_wait=keep, on_update=list(si.on_update)
                )
            idx += 1


def _build_nc(with_bv: bool, with_bo: bool):
    nc = bass.Bass("TRN2", target_bir_lowering=False, debug=False, num_devices=1)

    Qd = nc.dram_tensor("Q", [S, D], f32, kind="ExternalInput").ap()
    Kd = nc.dram_tensor("K", [S, D], f32, kind="ExternalInput").ap()
    Vd = nc.dram_tensor("V", [S, D], f32, kind="ExternalInput").ap()
    WQd = nc.dram_tensor("WQ", [D, D], f32, kind="ExternalInput").ap()
    WKd = nc.dram_tensor("WK", [D, D], f32, kind="ExternalInput").ap()
    WVd = nc.dram_tensor("WV", [D, D], f32, kind="ExternalInput").ap()
    WOd = nc.dram_tensor("WO", [D, D], f32, kind="ExternalInput").ap()
    bQd = nc.dram_tensor("bQ", [D], f32, kind="ExternalInput").ap()
    bKd = nc.dram_tensor("bK", [D], f32, kind="ExternalInput").ap()
    bVd = nc.dram_tensor("bV", [D], f32, kind="ExternalInput").ap()
    bOd = nc.dram_tensor("bO", [D], f32, kind="ExternalInput").ap()
    outd = nc.dram_tensor("out", [S, D], f32, kind="ExternalOutput").ap()

    with tile.TileContext(nc) as tc, ExitStack() as ctx:
        sb = ctx.enter_context(tc.tile_pool(name="sb", bufs=1))
        ps = ctx.enter_context(tc.tile_pool(name="ps", bufs=1, space="PSUM"))
        dramp = ctx.enter_context(tc.tile_pool(name="dram", bufs=1, space="DRAM"))

        # ---- constants -------------------------------------------------
        ident = sb.tile([P, P], f32, tag="ident", name="ident")
        make_identity(nc, ident)
        identb = sb.tile([P, P], bf16, tag="identb", name="identb")
        nc.vector.tensor_copy(identb, ident)
        bqk = sb.tile([P, 2 * NB], f32, tag="bqk", name="bqk")
        nc.sync.dma_start(bqk[:, 0:NB], bQd.rearrange("(db p) -> p db", p=P))
        nc.sync.dma_start(bqk[:, NB : 2 * NB], bKd.rearrange("(db p) -> p db", p=P))
        if with_bv:
            bvb = sb.tile([P, D], f32, tag="bvb", name="bvb")
            nc.sync.dma_start(bvb, bVd[None, :].broadcast_to([P, D]))
        if with_bo:
            bob = sb.tile([P, D], f32, tag="bob", name="bob")
            nc.sync.dma_start(bob, bOd[None, :].broadcast_to([P, D]))

        def big_tile(name):
            return ps.tile([P, 2, QC], f32, tag="big", bufs=2, name=name)

        def half_tile(name, tag="half"):
            return ps.tile([P, QC], f32, tag=tag, bufs=2, name=name)

        def wload(Wd, kb, chunk, tag="wper", bufs=16):
            """Stream a [128, 512] f32 weight strip and cast to bf16.
            wv (v_proj) and wo (o_proj) share one 16-slot ring: wo's allocs
            reuse wv's slots long after v_proj consumed them."""
            wstage = sb.tile([P, QC], f32, tag="wstage", bufs=3, name="wstage")
            nc.sync.dma_start(
                wstage, Wd[kb * P : (kb + 1) * P, chunk * QC : (chunk + 1) * QC]
            )
            wb = sb.tile([P, QC], bf16, tag=tag, bufs=bufs, name=tag)
            nc.vector.tensor_copy(wb, wstage)
            return wb

        def wload_d(Wd, kb, d):
            """One [128, 128] weight block (rows kb, cols d) for the per-pair
            q/k projections — loaded one iteration ahead of use."""
            wds = sb.tile([P, P], f32, tag="wds", bufs=6, name="wds")
            nc.sync.dma_start(
                wds, Wd[kb * P : (kb + 1) * P, d * P : (d + 1) * P]
            )
            wd = sb.tile([P, P], bf16, tag="wd", bufs=40, name="wd")
            nc.vector.tensor_copy(wd, wds)
            return wd

        def load_transpose(Xd, slot0):
            """HBM row-major -> feature-major bf16 tiles xt[db] (128 x 1024).
            All three tensors share one 16-slot ring; V uses slots 0-7 first,
            Q takes 8-15, K reuses 0-7 once v_proj has consumed them.
            Transpose psum rotates across all three pools (6-deep ring)."""
            xt = [
                sb.tile([P, S], bf16, tag="xt", bufs=16, name=f"xt{slot0 + i}")
                for i in range(NB)
            ]
            for sblk in range(NB):
                xn = sb.tile([P, D], f32, tag="xn", bufs=2, name="xn")
                nc.sync.dma_start(xn, Xd[sblk * P : (sblk + 1) * P, :])
                xnb = sb.tile([P, D], bf16, tag="xnb", bufs=2, name="xnb")
                nc.vector.tensor_copy(xnb, xn)
                for db in range(NB):
                    r = (sblk * NB + db) % 3
                    if r == 0:
                        tp = big_tile("tp")[:, 0, :]
                    elif r == 1:
                        tp = half_tile("tph")
                    else:
                        tp = half_tile("tpv", tag="vph")
                    tpb = tp[:, 0:P].bitcast(bf16)[:, 0:P]
                    nc.tensor.transpose(
                        tpb, xnb[:, db * P : (db + 1) * P], identb
                    )
                    dst = xt[db][:, sblk * P : (sblk + 1) * P]
                    if (sblk + db) % 2 == 0:
                        nc.vector.tensor_copy(dst, tpb)
                    else:
                        nc.scalar.activation(dst, tpb, AF.Copy)
            return xt

        # ---- preps: V first (v_proj consumes vt before k_prep reuses the
        # ring slots — k_prep MUST be emitted after v_proj or the PE queue
        # deadlocks) -----------------------------------------------------
        with nc.named_scope("v_prep"):
            vt = load_transpose(Vd, 0)
        with nc.named_scope("q_prep"):
            qt = load_transpose(Qd, 8)

        # ---- V projection -> vaug (row-major, head-major, 65th=ones) ----
        with nc.named_scope("v_proj"):
            vaug = [
                sb.tile([P, H * 65], bf16, tag="vaug", bufs=NB, name=f"vaug{i}")
                for i in range(NB)
            ]
            for sblk in range(NB):
                nc.vector.memset(
                    vaug[sblk].rearrange("p (h c) -> p h c", c=65)[:, :, 64:65],
                    1.0,
                )
            wv = [[wload(WVd, kb, c) for c in range(2)] for kb in range(NB)]
            for sblk in range(NB):
                acc = big_tile("vacc")
                for kb in range(NB):
                    for c in range(2):
                        nc.tensor.matmul(
                            acc[:, c, :],
                            vt[kb][:, sblk * P : (sblk + 1) * P],
                            wv[kb][c],
                            start=(kb == 0),
                            stop=(kb == NB - 1),
                        )
                for c in range(2):
                    if with_bv:
                        nc.vector.tensor_add(
                            acc[:, c, :], acc[:, c, :], bvb[:, c * QC : (c + 1) * QC]
                        )
                    dst = vaug[sblk].rearrange("p (h c) -> p h c", c=65)[
                        :, c * 8 : (c + 1) * 8, 0:64
                    ]
                    nc.scalar.activation(
                        dst,
                        acc[:, c, :].rearrange("p (h c) -> p h c", c=64),
                        AF.Relu,
                    )

        # k_prep reuses V's xt ring slots — must come after v_proj
        with nc.named_scope("k_prep"):
            kt = load_transpose(Kd, 0)

        # ---- per-pair pipeline pieces -----------------------------------
        def gen_projd(xt, wstrips, d, bias_base, tag, out):
            """Projection output block relu(W[:, dblk].T @ X^T + b) as a
            generator yielding after each 4-matmul chunk (rider stream).
            Appends the evicted feature-major bf16 ring tile to `out`."""
            acc0 = half_tile("pacc0")
            acc1 = half_tile("pacc1")
            for g in range(4):
                for kb in (2 * g, 2 * g + 1):
                    wt = wstrips[kb]
                    first, last = kb == 0, kb == NB - 1
                    nc.tensor.matmul(
                        acc0, wt, xt[kb][:, 0:QC], start=first, stop=last
                    )
                    nc.tensor.matmul(
                        acc1, wt, xt[kb][:, QC:S], start=first, stop=last
                    )
                yield
            xpt = sb.tile([P, S], bf16, tag=tag, bufs=2, name=tag)
            for c, acc in ((0, acc0), (1, acc1)):
                nc.vector.tensor_scalar(
                    out=xpt[:, c * QC : (c + 1) * QC],
                    in0=acc,
                    scalar1=bqk[:, bias_base + d : bias_base + d + 1],
                    scalar2=0.0,
                    op0=ALU.add,
                    op1=ALU.max,
                )
            out.append(xpt)

        def emit_scores_unit(d, qpt, kpt, riders):
            """Scores + exp for head pair d (even head on PE rows 0-63, odd
            on 64-127, concurrent in the array). After each key block the
            rider generators (PV of pair d-1, projections for pair d+1) get
            one step each, so the PE stays busy while the scores matmuls
            throttle on the exp-gated psum ring."""
            ptA = sb.tile([P, NB, 2, QC], bf16, tag="pt", bufs=4, name="ptA")
            ptB = sb.tile([P, NB, 2, QC], bf16, tag="pt", bufs=4, name="ptB")
            for kb in range(NB):
                ksl = slice(kb * P, (kb + 1) * P)
                spA = big_tile("spA")
                spB = big_tile("spB")
                for qc in range(2):
                    qsl = slice(qc * QC, (qc + 1) * QC)
                    nc.tensor.matmul(
                        spA[:, qc, :], kpt[0:DK, ksl], qpt[0:DK, qsl],
                        start=True, stop=True,
                    )
                for qc in range(2):
                    qsl = slice(qc * QC, (qc + 1) * QC)
                    nc.tensor.matmul(
                        spB[:, qc, :], kpt[DK:P, ksl], qpt[DK:P, qsl],
                        start=True, stop=True,
                    )
                nc.scalar.activation(ptA[:, kb, :, :], spA, AF.Exp, scale=0.03125)
                nc.scalar.activation(ptB[:, kb, :, :], spB, AF.Exp, scale=0.03125)
                for rd in riders:
                    if rd is not None:
                        next(rd, None)
            return ptA, ptB

        ot = [
            sb.tile([P, S], bf16, tag="ot", bufs=NB, name=f"ot{i}")
            for i in range(NB)
        ]

        def emit_pv_tail(h, vp):
            """Softmax division: reciprocal of the denominator row on a
            DRAM-spread layout + DMA broadcast + multiply (all off PE/ACT)."""
            dbq, off = h // 2, (h % 2) * DK
            for qc in range(2):
                qsl = slice(qc * QC, (qc + 1) * QC)
                stage = sb.tile([65, QC], f32, tag="stage", bufs=3, name="stage")
                nc.vector.tensor_copy(stage, vp[qc][0:65, :])
                scr = dramp.tile([1, QC], f32, tag="scr", bufs=6, name="scr")
                nc.gpsimd.dma_start(scr, stage[64:65, :])
                rcp = sb.tile([DK, NB], f32, tag="rcp", bufs=3, name="rcp")
                nc.gpsimd.dma_start(
                    rcp, scr.rearrange("o (a b) -> a (o b)", a=DK)
                )
                nc.vector.reciprocal(rcp, rcp)
                scr2 = dramp.tile([1, QC], f32, tag="scr2", bufs=6, name="scr2")
                nc.gpsimd.dma_start(
                    scr2.rearrange("o (a b) -> a (o b)", a=DK), rcp
                )
                bc = sb.tile([DK, QC], f32, tag="bc", bufs=2, name="bc")
                nc.gpsimd.dma_start(bc, scr2.broadcast_to([DK, QC]))
                if off == 0:
                    nc.vector.tensor_mul(ot[dbq][0:DK, qsl], stage[0:DK, :], bc)
                else:
                    tmp = sb.tile([DK, QC], bf16, tag="tmp", bufs=2, name="tmp")
                    nc.vector.tensor_mul(tmp, stage[0:DK, :], bc)
                    nc.gpsimd.dma_start(ot[dbq][DK:P, qsl], tmp)

        def gen_pv_pair(d, ptA, ptB):
            """PV + division for head pair (2d, 2d+1), both q-chunks, yielded
            in 8 groups of 4 matmuls for interleaving with other PE work."""
            for hl, ptX in ((0, ptA), (1, ptB)):
                h = 2 * d + hl
                vp = [half_tile(f"vp{h}_{qc}", tag="vph") for qc in range(2)]
                for g in range(4):
                    for kb in (2 * g, 2 * g + 1):
                        for qc in range(2):
                            nc.tensor.matmul(
                                vp[qc][0:65, :],
                                vaug[kb][:, h * 65 : (h + 1) * 65],
                                ptX[:, kb, qc, :],
                                start=(kb == 0),
                                stop=(kb == NB - 1),
                            )
                    yield
                emit_pv_tail(h, vp)

        # ---- pipeline: preamble proj(0), then fused iterations ----------
        wnext = [
            [wload_d(WKd, kb, 0) for kb in range(NB)],
            [wload_d(WQd, kb, 0) for kb in range(NB)],
        ]
        with nc.named_scope("proj0"):
            cur = []
            for _ in itertools.chain(
                gen_projd(kt, wnext[0], 0, NB, "kpt", cur),
                gen_projd(qt, wnext[1], 0, 0, "qpt", cur),
            ):
                pass
            kpt_c, qpt_c = cur

        wo = []
        pend = None
        for d in range(NB):
            with nc.named_scope(f"it{d}"):
                if d + 1 < NB:
                    wnext = [
                        [wload_d(WKd, kb, d + 1) for kb in range(NB)],
                        [wload_d(WQd, kb, d + 1) for kb in range(NB)],
                    ]
                    nxt = []
                    g_proj = itertools.chain(
                        gen_projd(kt, wnext[0], d + 1, NB, "kpt", nxt),
                        gen_projd(qt, wnext[1], d + 1, 0, "qpt", nxt),
                    )
                else:
                    nxt, g_proj = None, None
                g_pv = gen_pv_pair(d - 1, *pend) if pend is not None else None
                if g_pv is not None:
                    next(g_pv, None)  # prime one group for iteration start
                pend = emit_scores_unit(d, qpt_c, kpt_c, [g_pv, g_proj])
                for g in (g_pv, g_proj):
                    if g is not None:
                        for _ in g:
                            pass
                if nxt is not None:
                    kpt_c, qpt_c = nxt
                if 4 <= d < 7:
                    # spread the o_proj weight loads across iterations 4-6
                    wo.append([wload(WOd, 2 * (d - 4), c) for c in range(2)])
                    wo.append(
                        [wload(WOd, 2 * (d - 4) + 1, c) for c in range(2)]
                    )
                    if d == 6:
                        wo.append([wload(WOd, 6, c) for c in range(2)])
                        wo.append([wload(WOd, 7, c) for c in range(2)])

        # ---- tail: o_proj chains (db 0-6 open early; db7 lands last) ----
        def oproj_open(sblk, accs, rider=None):
            for db in range(NB - 1):
                for c in range(2):
                    nc.tensor.matmul(
                        accs[c],
                        ot[db][:, sblk * P : (sblk + 1) * P],
                        wo[db][c],
                        start=(db == 0),
                        stop=False,
                    )
                if rider is not None and db % 2 == 1:
                    next(rider, None)
            return accs

        def oproj_close(sblk, accs):
            for c in range(2):
                nc.tensor.matmul(
                    accs[c],
                    ot[NB - 1][:, sblk * P : (sblk + 1) * P],
                    wo[NB - 1][c],
                    start=False,
                    stop=True,
                )
            for c in range(2):
                if with_bo:
                    nc.vector.tensor_add(
                        accs[c], accs[c], bob[:, c * QC : (c + 1) * QC]
                    )
                o = sb.tile([P, QC], f32, tag="obuf", bufs=2, name="obuf")
                nc.scalar.activation(o, accs[c], AF.Relu)
                nc.sync.dma_start(
                    outd[sblk * P : (sblk + 1) * P, c * QC : (c + 1) * QC], o
                )

        def chain_accs(sblk):
            """Chain accumulators: sblk 0,1,4,5 use the two big-pool tiles;
            2,3,6,7 pair a half tile with a vph tile (both free at the tail)."""
            if sblk % 4 < 2:
                t = big_tile(f"oacc{sblk}")
                return [t[:, 0, :], t[:, 1, :]]
            return [half_tile(f"oh{sblk}"), half_tile(f"ov{sblk}", tag="vph")]

        with nc.named_scope("o_proj"):
            gpv = gen_pv_pair(NB - 1, *pend)
            next(gpv, None)
            chains = {}
            # big-pool chains ride the last PV pair; the split (half+vph)
            # chains may only allocate after gpv fully drains, else their
            # matmuls wait on vph stage-copies that sit later in the PE FIFO
            for sblk in range(2):
                chains[sblk] = oproj_open(sblk, chain_accs(sblk), rider=gpv)
            for _ in gpv:
                pass
            for sblk in range(2, 4):
                chains[sblk] = oproj_open(sblk, chain_accs(sblk))
            for sblk in range(4, NB):
                oproj_close(sblk - 4, chains.pop(sblk - 4))
                chains[sblk] = oproj_open(sblk, chain_accs(sblk))
            for sblk in range(NB - 4, NB):
                oproj_close(sblk, chains.pop(sblk))

    _split_wide_waits(nc)
    return nc


_NC_CACHE = {}


def kernel(Q, K, V, WQ, bQ, WK, bK, WV, bV, WO, bO, h):
    Q, K, V = (np.ascontiguousarray(np.asarray(x, np.float32)) for x in (Q, K, V))
    WQ, WK, WV, WO = (
        np.ascontiguousarray(np.asarray(x, np.float32)) for x in (WQ, WK, WV, WO)
    )
    bQ, bK, bV, bO = (
        np.ascontiguousarray(np.asarray(x, np.float32)) for x in (bQ, bK, bV, bO)
    )
    h = int(np.asarray(h))
    assert h == H, f"kernel specialized for h=16, got {h}"
    B = Q.shape[0]
    assert Q.shape == (B, S, D) and B == N_CORES

    key = (bool(np.any(bV)), bool(np.any(bO)))
    if key not in _NC_CACHE:
        _NC_CACHE[key] = _build_nc(*key)
    nc = _NC_CACHE[key]

    in_maps = [
        {
            "Q": Q[b], "K": K[b], "V": V[b],
            "WQ": WQ, "WK": WK, "WV": WV, "WO": WO,
            "bQ": bQ, "bK": bK, "bV": bV, "bO": bO,
        }
        for b in range(B)
    ]
    trace = os.environ.get("BASS_MHA_TRACE") == "1"
    res = run_bass_kernel_spmd(
        nc, in_maps, core_ids=list(range(N_CORES)), trace=trace
    )
    if trace:
        kernel.last_results = res
    return np.stack([res.results[b]["out"] for b in range(B)], axis=0)


# revision 32
# speedup vs baseline: 1.0800x; 1.0251x over previous
"""Trainium2 Bass kernel for nn_MultiHeadAttention (B=8, S=1024, D=1024, h=16).

Sharding: pure data-parallel over batch — each of the 8 NeuronCores computes
the full MHA for one batch element. No collectives.

Per-core design (bf16 matmul operands, fp32 PSUM accumulation). The softmax
exp chain on ScalarE (16 ACTIVATEs x ~1.1us per head pair = ~16.3us/pair) is
the steady-state floor, so the whole kernel is built as a software pipeline
over head pairs that keeps ScalarE saturated:

  iteration d: scores(d) matmuls (throttled to exp rate by the psum ring)
  with TWO rider streams interleaved per key block — PV(d-1) matmul groups
  and the q/k projection chunks for pair d+1. PE, ScalarE and VectorE all
  stay ~90% busy at a ~17-19us pair cadence.

PSUM is split into three pools so the long-lived projection accumulators
never sit in the scores ring's reuse path:
  "big"  2 x [128,2,512] (4 banks): scores spA/spB, v_proj accs, transposes
  "half" 2 x [128,512]   (2 banks): q/k projection accumulators
  "vph"  2 x [128,512]   (2 banks): PV accumulators (65 rows: 64 V + ones)

Softmax denominators ride as a 65th ones-column inside the PV stationary;
the division runs off a SBUF staging copy via a DRAM-spread reciprocal
broadcast on GpSimd DMAs (off the PE/ACT critical path).

o_proj opens db0-6 accumulation chains before the last pair's division
lands (ot[7] is the only missing input), then closes them with the db7
contribution + relu as it arrives.
"""
import itertools
import os
from contextlib import ExitStack

import numpy as np

import concourse.bass as bass
import concourse.tile as tile
from concourse import mybir
from concourse.bass_utils import run_bass_kernel_spmd
from concourse.masks import make_identity

f32 = mybir.dt.float32
bf16 = mybir.dt.bfloat16
AF = mybir.ActivationFunctionType
ALU = mybir.AluOpType

S = 1024
D = 1024
H = 16
DK = 64
P = 128
NB = D // P  # 8 blocks
QC = 512
N_CORES = 8


def _split_wide_waits(nc, max_waits=1):
    """This walrus build rejects instructions carrying more than one
    semaphore wait; move excess waits onto NoOp carriers inserted before
    the offending instruction on the same engine."""
    for bb in nc.m.functions[0].blocks:
        idx = 0
        while idx < len(bb.instructions):
            ins = bb.instructions[idx]
            si = ins.sync_info
            if si is not None and si.on_wait and len(si.on_wait) > max_waits:
                waits = list(si.on_wait)
                rest, keep = waits[:-max_waits], waits[-max_waits:]
                for j in range(0, len(rest), max_waits):
                    nop = mybir.InstNoOp(
                        name=f"I-waitsplit-{nc.next_id()}",
                        engine=ins.engine,
                        ins=[],
                        outs=[],
                    )
                    nop.sync_info = mybir.SyncInfo(
                        on_wait=rest[j : j + max_waits], on_update=[]
                    )
                    nc.register_instruction(nop)
                    bb.instructions.insert(idx, nop)
                    idx += 1
                ins.sync_info = mybir.SyncInfo(
                    on_wait=keep, on_update=list(si.on_update)
                )
            idx += 1


def _build_nc(with_bv: bool, with_bo: bool):
    nc = bass.Bass("TRN2", target_bir_lowering=False, debug=False, num_devices=1)

    Qd = nc.dram_tensor("Q", [S, D], f32, kind="ExternalInput").ap()
    Kd = nc.dram_tensor("K", [S, D], f32, kind="ExternalInput").ap()
    Vd = nc.dram_tensor("V", [S, D], f32, kind="ExternalInput").ap()
    WQd = nc.dram_tensor("WQ", [D, D], f32, kind="ExternalInput").ap()
    WKd = nc.dram_tensor("WK", [D, D], f32, kind="ExternalInput").ap()
    WVd = nc.dram_tensor("WV", [D, D], f32, kind="ExternalInput").ap()
    WOd = nc.dram_tensor("WO", [D, D], f32, kind="ExternalInput").ap()
    bQd = nc.dram_tensor("bQ", [D], f32, kind="ExternalInput").ap()
    bKd = nc.dram_tensor("bK", [D], f32, kind="ExternalInput").ap()
    bVd = nc.dram_tensor("bV", [D], f32, kind="ExternalInput").ap()
    bOd = nc.dram_tensor("bO", [D], f32, kind="ExternalInput").ap()
    outd = nc.dram_tensor("out", [S, D], f32, kind="ExternalOutput").ap()

    with tile.TileContext(nc) as tc, ExitStack() as ctx:
        sb = ctx.enter_context(tc.tile_pool(name="sb", bufs=1))
        ps = ctx.enter_context(tc.tile_pool(name="ps", bufs=1, space="PSUM"))
        dramp = ctx.enter_context(tc.tile_pool(name="dram", bufs=1, space="DRAM"))

        # ---- constants -------------------------------------------------
        ident = sb.tile([P, P], f32, tag="ident", name="ident")
        make_identity(nc, ident)
        identb = sb.tile([P, P], bf16, tag="identb", name="identb")
        nc.vector.tensor_copy(identb, ident)
        bqk = sb.tile([P, 2 * NB], f32, tag="bqk", name="bqk")
        nc.sync.dma_start(bqk[:, 0:NB], bQd.rearrange("(db p) -> p db", p=P))
        nc.sync.dma_start(bqk[:, NB : 2 * NB], bKd.rearrange("(db p) -> p db", p=P))
        if with_bv:
            bvb = sb.tile([P, D], f32, tag="bvb", name="bvb")
            nc.sync.dma_start(bvb, bVd[None, :].broadcast_to([P, D]))
        if with_bo:
            bob = sb.tile([P, D], f32, tag="bob", name="bob")
            nc.sync.dma_start(bob, bOd[None, :].broadcast_to([P, D]))

        def big_tile(name):
            return ps.tile([P, 2, QC], f32, tag="big", bufs=2, name=name)

        def half_tile(name, tag="half"):
            return ps.tile([P, QC], f32, tag=tag, bufs=2, name=name)

        def wload(Wd, kb, chunk, tag="wper", bufs=16):
            """Stream a [128, 512] f32 weight strip and cast to bf16.
            wv (v_proj) and wo (o_proj) share one 16-slot ring: wo's allocs
            reuse wv's slots long after v_proj consumed them."""
            wstage = sb.tile([P, QC], f32, tag="wstage", bufs=3, name="wstage")
            nc.sync.dma_start(
                wstage, Wd[kb * P : (kb + 1) * P, chunk * QC : (chunk + 1) * QC]
            )
            wb = sb.tile([P, QC], bf16, tag=tag, bufs=bufs, name=tag)
            nc.vector.tensor_copy(wb, wstage)
            return wb

        def wload_d(Wd, kb, d):
            """One [128, 128] weight block (rows kb, cols d) for the per-pair
            q/k projections — loaded one iteration ahead of use."""
            wds = sb.tile([P, P], f32, tag="wds", bufs=6, name="wds")
            nc.sync.dma_start(
                wds, Wd[kb * P : (kb + 1) * P, d * P : (d + 1) * P]
            )
            wd = sb.tile([P, P], bf16, tag="wd", bufs=40, name="wd")
            nc.vector.tensor_copy(wd, wds)
            return wd

        def load_transpose(Xd, slot0):
            """HBM row-major -> feature-major bf16 tiles xt[db] (128 x 1024).
            All three tensors share one 16-slot ring; V uses slots 0-7 first,
            Q takes 8-15, K reuses 0-7 once v_proj has consumed them.
            Transpose psum rotates across all three pools (6-deep ring)."""
            xt = [
                sb.tile([P, S], bf16, tag="xt", bufs=16, name=f"xt{slot0 + i}")
                for i in range(NB)
            ]
            for sblk in range(NB):
                xn = sb.tile([P, D], f32, tag="xn", bufs=2, name="xn")
                nc.sync.dma_start(xn, Xd[sblk * P : (sblk + 1) * P, :])
                xnb = sb.tile([P, D], bf16, tag="xnb", bufs=2, name="xnb")
                nc.vector.tensor_copy(xnb, xn)
                for db in range(NB):
                    r = (sblk * NB + db) % 3
                    if r == 0:
                        tp = big_tile("tp")[:, 0, :]
                    elif r == 1:
                        tp = half_tile("tph")
                    else:
                        tp = half_tile("tpv", tag="vph")
                    tpb = tp[:, 0:P].bitcast(bf16)[:, 0:P]
                    nc.tensor.transpose(
                        tpb, xnb[:, db * P : (db + 1) * P], identb
                    )
                    dst = xt[db][:, sblk * P : (sblk + 1) * P]
                    if (sblk + db) % 2 == 0:
                        nc.vector.tensor_copy(dst, tpb)
                    else:
                        nc.scalar.activation(dst, tpb, AF.Copy)
            return xt

        # ---- preps: V first (v_proj consumes vt before k_prep reuses the
        # ring slots — k_prep MUST be emitted after v_proj or the PE queue
        # deadlocks) -----------------------------------------------------
        with nc.named_scope("v_prep"):
            vt = load_transpose(Vd, 0)
        with nc.named_scope("q_prep"):
            qt = load_transpose(Qd, 8)

        # ---- V projection -> vaug (row-major, head-major, 65th=ones) ----
        with nc.named_scope("v_proj"):
            vaug = [
                sb.tile([P, H * 65], bf16, tag="vaug", bufs=NB, name=f"vaug{i}")
                for i in range(NB)
            ]
            for sblk in range(NB):
                nc.vector.memset(
                    vaug[sblk].rearrange("p (h c) -> p h c", c=65)[:, :, 64:65],
                    1.0,
                )
            wv = [[wload(WVd, kb, c) for c in range(2)] for kb in range(NB)]
            for sblk in range(NB):
                acc = big_tile("vacc")
                for kb in range(NB):
                    for c in range(2):
                        nc.tensor.matmul(
                            acc[:, c, :],
                            vt[kb][:, sblk * P : (sblk + 1) * P],
                            wv[kb][c],
                            start=(kb == 0),
                            stop=(kb == NB - 1),
                        )
                for c in range(2):
                    if with_bv:
                        nc.vector.tensor_add(
                            acc[:, c, :], acc[:, c, :], bvb[:, c * QC : (c + 1) * QC]
                        )
                    dst = vaug[sblk].rearrange("p (h c) -> p h c", c=65)[
                        :, c * 8 : (c + 1) * 8, 0:64
                    ]
                    nc.scalar.activation(
                        dst,
                        acc[:, c, :].rearrange("p (h c) -> p h c", c=64),
                        AF.Relu,
                    )

        # k_prep reuses V's xt ring slots — must come after v_proj
        with nc.named_scope("k_prep"):
            kt = load_transpose(Kd, 0)

        # ---- per-pair pipeline pieces -----------------------------------
        def gen_projd(xt, wstrips, d, bias_base, tag, out):
            """Projection output block relu(W[:, dblk].T @ X^T + b) as a
            generator yielding after each 4-matmul chunk (rider stream).
            Appends the evicted feature-major bf16 ring tile to `out`."""
            acc0 = half_tile("pacc0")
            acc1 = half_tile("pacc1")
            for g in range(4):
                for kb in (2 * g, 2 * g + 1):
                    wt = wstrips[kb]
                    first, last = kb == 0, kb == NB - 1
                    nc.tensor.matmul(
                        acc0, wt, xt[kb][:, 0:QC], start=first, stop=last
                    )
                    nc.tensor.matmul(
                        acc1, wt, xt[kb][:, QC:S], start=first, stop=last
                    )
                yield
            xpt = sb.tile([P, S], bf16, tag=tag, bufs=2, name=tag)
            for c, acc in ((0, acc0), (1, acc1)):
                nc.vector.tensor_scalar(
                    out=xpt[:, c * QC : (c + 1) * QC],
                    in0=acc,
                    scalar1=bqk[:, bias_base + d : bias_base + d + 1],
                    scalar2=0.0,
                    op0=ALU.add,
                    op1=ALU.max,
                )
            out.append(xpt)

        def emit_scores_unit(d, qpt, kpt, riders):
            """Scores + exp for head pair d (even head on PE rows 0-63, odd
            on 64-127, concurrent in the array). After each key block the
            rider generators (PV of pair d-1, projections for pair d+1) get
            one step each, so the PE stays busy while the scores matmuls
            throttle on the exp-gated psum ring."""
            ptA = sb.tile([P, NB, 2, QC], bf16, tag="pt", bufs=4, name="ptA")
            ptB = sb.tile([P, NB, 2, QC], bf16, tag="pt", bufs=4, name="ptB")
            for kb in range(NB):
                ksl = slice(kb * P, (kb + 1) * P)
                spA = big_tile("spA")
                spB = big_tile("spB")
                for qc in range(2):
                    qsl = slice(qc * QC, (qc + 1) * QC)
                    nc.tensor.matmul(
                        spA[:, qc, :], kpt[0:DK, ksl], qpt[0:DK, qsl],
                        start=True, stop=True,
                    )
                for qc in range(2):
                    qsl = slice(qc * QC, (qc + 1) * QC)
                    nc.tensor.matmul(
                        spB[:, qc, :], kpt[DK:P, ksl], qpt[DK:P, qsl],
                        start=True, stop=True,
                    )
                nc.scalar.activation(ptA[:, kb, :, :], spA, AF.Exp, scale=0.03125)
                nc.scalar.activation(ptB[:, kb, :, :], spB, AF.Exp, scale=0.03125)
                for rd in riders:
                    if rd is not None:
                        next(rd, None)
            return ptA, ptB

        ot = [
            sb.tile([P, S], bf16, tag="ot", bufs=NB, name=f"ot{i}")
            for i in range(NB)
        ]

        def emit_pv_tail(h, vp):
            """Softmax division: reciprocal of the denominator row on a
            DRAM-spread layout + DMA broadcast + multiply (all off PE/ACT)."""
            dbq, off = h // 2, (h % 2) * DK
            for qc in range(2):
                qsl = slice(qc * QC, (qc + 1) * QC)
                stage = sb.tile([65, QC], f32, tag="stage", bufs=3, name="stage")
                nc.vector.tensor_copy(stage, vp[qc][0:65, :])
                scr = dramp.tile([1, QC], f32, tag="scr", bufs=6, name="scr")
                nc.gpsimd.dma_start(scr, stage[64:65, :])
                rcp = sb.tile([DK, NB], f32, tag="rcp", bufs=3, name="rcp")
                nc.gpsimd.dma_start(
                    rcp, scr.rearrange("o (a b) -> a (o b)", a=DK)
                )
                nc.vector.reciprocal(rcp, rcp)
                scr2 = dramp.tile([1, QC], f32, tag="scr2", bufs=6, name="scr2")
                nc.gpsimd.dma_start(
                    scr2.rearrange("o (a b) -> a (o b)", a=DK), rcp
                )
                bc = sb.tile([DK, QC], f32, tag="bc", bufs=2, name="bc")
                nc.gpsimd.dma_start(bc, scr2.broadcast_to([DK, QC]))
                if off == 0:
                    nc.vector.tensor_mul(ot[dbq][0:DK, qsl], stage[0:DK, :], bc)
                else:
                    tmp = sb.tile([DK, QC], bf16, tag="tmp", bufs=2, name="tmp")
                    nc.vector.tensor_mul(tmp, stage[0:DK, :], bc)
                    nc.gpsimd.dma_start(ot[dbq][DK:P, qsl], tmp)

        def gen_pv_pair(d, ptA, ptB):
            """PV + division for head pair (2d, 2d+1), both q-chunks, yielded
            in 8 groups of 4 matmuls for interleaving with other PE work."""
            for hl, ptX in ((0, ptA), (1, ptB)):
                h = 2 * d + hl
                vp = [half_tile(f"vp{h}_{qc}", tag="vph") for qc in range(2)]
                for g in range(4):
                    for kb in (2 * g, 2 * g + 1):
                        for qc in range(2):
                            nc.tensor.matmul(
                                vp[qc][0:65, :],
                                vaug[kb][:, h * 65 : (h + 1) * 65],
                                ptX[:, kb, qc, :],
                                start=(kb == 0),
                                stop=(kb == NB - 1),
                            )
                    yield
                emit_pv_tail(h, vp)

        # ---- pipeline: preamble proj(0), then fused iterations ----------
        wnext = [
            [wload_d(WKd, kb, 0) for kb in range(NB)],
            [wload_d(WQd, kb, 0) for kb in range(NB)],
        ]
        with nc.named_scope("proj0"):
            cur = []
            for _ in itertools.chain(
                gen_projd(kt, wnext[0], 0, NB, "kpt", cur),
                gen_projd(qt, wnext[1], 0, 0, "qpt", cur),
            ):
                pass
            kpt_c, qpt_c = cur

        wo = []
        pend = None
        for d in range(NB):
            with nc.named_scope(f"it{d}"):
                if d + 1 < NB:
                    wnext = [
                        [wload_d(WKd, kb, d + 1) for kb in range(NB)],
                        [wload_d(WQd, kb, d + 1) for kb in range(NB)],
                    ]
                    nxt = []
                    g_proj = itertools.chain(
                        gen_projd(kt, wnext[0], d + 1, NB, "kpt", nxt),
                        gen_projd(qt, wnext[1], d + 1, 0, "qpt", nxt),
                    )
                else:
                    nxt, g_proj = None, None
                g_pv = gen_pv_pair(d - 1, *pend) if pend is not None else None
                if g_pv is not None:
                    next(g_pv, None)  # prime one group for iteration start
                pend = emit_scores_unit(d, qpt_c, kpt_c, [g_pv, g_proj])
                for g in (g_pv, g_proj):
                    if g is not None:
                        for _ in g:
                            pass
                if nxt is not None:
                    kpt_c, qpt_c = nxt
                if 4 <= d < 7:
                    # spread the o_proj weight loads across iterations 4-6
                    wo.append([wload(WOd, 2 * (d - 4), c) for c in range(2)])
                    wo.append(
                        [wload(WOd, 2 * (d - 4) + 1, c) for c in range(2)]
                    )
                    if d == 6:
                        wo.append([wload(WOd, 6, c) for c in range(2)])
                        wo.append([wload(WOd, 7, c) for c in range(2)])

        # ---- tail: o_proj chains (db 0-6 open early; db7 lands last) ----
        def oproj_open(sblk, accs, rider=None):
            for db in range(NB - 1):
                for c in range(2):
                    nc.tensor.matmul(
                        accs[c],
                        ot[db][:, sblk * P : (sblk + 1) * P],
                        wo[db][c],
                        start=(db == 0),
                        stop=False,
                    )
                if rider is not None and db % 2 == 1:
                    next(rider, None)
            return accs

        def oproj_close(sblk, accs):
            for c in range(2):
                nc.tensor.matmul(
                    accs[c],
                    ot[NB - 1][:, sblk * P : (sblk + 1) * P],
                    wo[NB - 1][c],
                    start=False,
                    stop=True,
                )
            for c in range(2):
                if with_bo:
                    nc.vector.tensor_add(
                        accs[c], accs[c], bob[:, c * QC : (c + 1) * QC]
                    )
                # 4-deep obuf ring + SWDGE queue (idle at the tail, unlike
                # Sync): keeps the relu evictions from serializing on the
                # out-DMA completions
                o = sb.tile([P, QC], f32, tag="obuf", bufs=4, name="obuf")
                nc.scalar.activation(o, accs[c], AF.Relu)
                nc.gpsimd.dma_start(
                    outd[sblk * P : (sblk + 1) * P, c * QC : (c + 1) * QC], o
                )

        def chain_accs(sblk):
            """Chain accumulators: sblk 0,1,4,5 use the two big-pool tiles;
            2,3,6,7 pair a half tile with a vph tile (both free at the tail)."""
            if sblk % 4 < 2:
                t = big_tile(f"oacc{sblk}")
                return [t[:, 0, :], t[:, 1, :]]
            return [half_tile(f"oh{sblk}"), half_tile(f"ov{sblk}", tag="vph")]

        with nc.named_scope("o_proj"):
            gpv = gen_pv_pair(NB - 1, *pend)
            next(gpv, None)
            chains = {}
            # big-pool chains ride the last PV pair; the split (half+vph)
            # chains may only allocate after gpv fully drains, else their
            # matmuls wait on vph stage-copies that sit later in the PE FIFO
            for sblk in range(2):
                chains[sblk] = oproj_open(sblk, chain_accs(sblk), rider=gpv)
            for _ in gpv:
                pass
            for sblk in range(2, 4):
                chains[sblk] = oproj_open(sblk, chain_accs(sblk))
            for sblk in range(4, NB):
                oproj_close(sblk - 4, chains.pop(sblk - 4))
                chains[sblk] = oproj_open(sblk, chain_accs(sblk))
            for sblk in range(NB - 4, NB):
                oproj_close(sblk, chains.pop(sblk))

    _split_wide_waits(nc)
    return nc


_NC_CACHE = {}


def kernel(Q, K, V, WQ, bQ, WK, bK, WV, bV, WO, bO, h):
    Q, K, V = (np.ascontiguousarray(np.asarray(x, np.float32)) for x in (Q, K, V))
    WQ, WK, WV, WO = (
        np.ascontiguousarray(np.asarray(x, np.float32)) for x in (WQ, WK, WV, WO)
    )
    bQ, bK, bV, bO = (
        np.ascontiguousarray(np.asarray(x, np.float32)) for x in (bQ, bK, bV, bO)
    )
    h = int(np.asarray(h))
    assert h == H, f"kernel specialized for h=16, got {h}"
    B = Q.shape[0]
    assert Q.shape == (B, S, D) and B == N_CORES

    key = (bool(np.any(bV)), bool(np.any(bO)))
    if key not in _NC_CACHE:
        _NC_CACHE[key] = _build_nc(*key)
    nc = _NC_CACHE[key]

    in_maps = [
        {
            "Q": Q[b], "K": K[b], "V": V[b],
            "WQ": WQ, "WK": WK, "WV": WV, "WO": WO,
            "bQ": bQ, "bK": bK, "bV": bV, "bO": bO,
        }
        for b in range(B)
    ]
    trace = os.environ.get("BASS_MHA_TRACE") == "1"
    res = run_bass_kernel_spmd(
        nc, in_maps, core_ids=list(range(N_CORES)), trace=trace
    )
    if trace:
        kernel.last_results = res
    return np.stack([res.results[b]["out"] for b in range(B)], axis=0)


# revision 33
# speedup vs baseline: 1.1011x; 1.0195x over previous
"""Trainium2 Bass kernel for nn_MultiHeadAttention (B=8, S=1024, D=1024, h=16).

Sharding: pure data-parallel over batch — each of the 8 NeuronCores computes
the full MHA for one batch element. No collectives.

Per-core design (bf16 matmul operands, fp32 PSUM accumulation). The softmax
exp chain on ScalarE (16 ACTIVATEs x ~1.1us per head pair = ~16.3us/pair) is
the steady-state floor, so the whole kernel is built as a software pipeline
over head pairs that keeps ScalarE saturated:

  iteration d: scores(d) matmuls (throttled to exp rate by the psum ring)
  with TWO rider streams interleaved per key block — PV(d-1) matmul groups
  and the q/k projection chunks for pair d+1. PE, ScalarE and VectorE all
  stay ~90% busy at a ~17-19us pair cadence.

PSUM is split into three pools so the long-lived projection accumulators
never sit in the scores ring's reuse path:
  "big"  2 x [128,2,512] (4 banks): scores spA/spB, v_proj accs, transposes
  "half" 2 x [128,512]   (2 banks): q/k projection accumulators
  "vph"  2 x [128,512]   (2 banks): PV accumulators (65 rows: 64 V + ones)

Softmax denominators ride as a 65th ones-column inside the PV stationary;
the division runs off a SBUF staging copy via a DRAM-spread reciprocal
broadcast on GpSimd DMAs (off the PE/ACT critical path).

o_proj opens db0-6 accumulation chains before the last pair's division
lands (ot[7] is the only missing input), then closes them with the db7
contribution + relu as it arrives.
"""
import itertools
import os
from contextlib import ExitStack

import numpy as np

import concourse.bass as bass
import concourse.tile as tile
from concourse import mybir
from concourse.bass_utils import run_bass_kernel_spmd
from concourse.masks import make_identity

f32 = mybir.dt.float32
bf16 = mybir.dt.bfloat16
AF = mybir.ActivationFunctionType
ALU = mybir.AluOpType

S = 1024
D = 1024
H = 16
DK = 64
P = 128
NB = D // P  # 8 blocks
QC = 512
N_CORES = 8


def _split_wide_waits(nc, max_waits=1):
    """This walrus build rejects instructions carrying more than one
    semaphore wait; move excess waits onto NoOp carriers inserted before
    the offending instruction on the same engine."""
    for bb in nc.m.functions[0].blocks:
        idx = 0
        while idx < len(bb.instructions):
            ins = bb.instructions[idx]
            si = ins.sync_info
            if si is not None and si.on_wait and len(si.on_wait) > max_waits:
                waits = list(si.on_wait)
                rest, keep = waits[:-max_waits], waits[-max_waits:]
                for j in range(0, len(rest), max_waits):
                    nop = mybir.InstNoOp(
                        name=f"I-waitsplit-{nc.next_id()}",
                        engine=ins.engine,
                        ins=[],
                        outs=[],
                    )
                    nop.sync_info = mybir.SyncInfo(
                        on_wait=rest[j : j + max_waits], on_update=[]
                    )
                    nc.register_instruction(nop)
                    bb.instructions.insert(idx, nop)
                    idx += 1
                ins.sync_info = mybir.SyncInfo(
                    on_wait=keep, on_update=list(si.on_update)
                )
            idx += 1


def _build_nc(with_bv: bool, with_bo: bool):
    nc = bass.Bass("TRN2", target_bir_lowering=False, debug=False, num_devices=1)

    Qd = nc.dram_tensor("Q", [S, D], f32, kind="ExternalInput").ap()
    Kd = nc.dram_tensor("K", [S, D], f32, kind="ExternalInput").ap()
    Vd = nc.dram_tensor("V", [S, D], f32, kind="ExternalInput").ap()
    WQd = nc.dram_tensor("WQ", [D, D], f32, kind="ExternalInput").ap()
    WKd = nc.dram_tensor("WK", [D, D], f32, kind="ExternalInput").ap()
    WVd = nc.dram_tensor("WV", [D, D], f32, kind="ExternalInput").ap()
    WOd = nc.dram_tensor("WO", [D, D], f32, kind="ExternalInput").ap()
    bQd = nc.dram_tensor("bQ", [D], f32, kind="ExternalInput").ap()
    bKd = nc.dram_tensor("bK", [D], f32, kind="ExternalInput").ap()
    bVd = nc.dram_tensor("bV", [D], f32, kind="ExternalInput").ap()
    bOd = nc.dram_tensor("bO", [D], f32, kind="ExternalInput").ap()
    outd = nc.dram_tensor("out", [S, D], f32, kind="ExternalOutput").ap()

    with tile.TileContext(nc) as tc, ExitStack() as ctx:
        sb = ctx.enter_context(tc.tile_pool(name="sb", bufs=1))
        ps = ctx.enter_context(tc.tile_pool(name="ps", bufs=1, space="PSUM"))
        dramp = ctx.enter_context(tc.tile_pool(name="dram", bufs=1, space="DRAM"))

        # ---- constants -------------------------------------------------
        ident = sb.tile([P, P], f32, tag="ident", name="ident")
        make_identity(nc, ident)
        identb = sb.tile([P, P], bf16, tag="identb", name="identb")
        nc.vector.tensor_copy(identb, ident)
        bqk = sb.tile([P, 2 * NB], f32, tag="bqk", name="bqk")
        nc.sync.dma_start(bqk[:, 0:NB], bQd.rearrange("(db p) -> p db", p=P))
        nc.sync.dma_start(bqk[:, NB : 2 * NB], bKd.rearrange("(db p) -> p db", p=P))
        if with_bv:
            bvb = sb.tile([P, D], f32, tag="bvb", name="bvb")
            nc.sync.dma_start(bvb, bVd[None, :].broadcast_to([P, D]))
        if with_bo:
            bob = sb.tile([P, D], f32, tag="bob", name="bob")
            nc.sync.dma_start(bob, bOd[None, :].broadcast_to([P, D]))

        def big_tile(name):
            return ps.tile([P, 2, QC], f32, tag="big", bufs=2, name=name)

        def half_tile(name, tag="half"):
            return ps.tile([P, QC], f32, tag=tag, bufs=2, name=name)

        def wload(Wd, kb, chunk, tag="wper", bufs=16):
            """Stream a [128, 512] f32 weight strip and cast to bf16.
            wv (v_proj) and wo (o_proj) share one 16-slot ring: wo's allocs
            reuse wv's slots long after v_proj consumed them."""
            wstage = sb.tile([P, QC], f32, tag="wstage", bufs=3, name="wstage")
            nc.sync.dma_start(
                wstage, Wd[kb * P : (kb + 1) * P, chunk * QC : (chunk + 1) * QC]
            )
            wb = sb.tile([P, QC], bf16, tag=tag, bufs=bufs, name=tag)
            nc.vector.tensor_copy(wb, wstage)
            return wb

        def wload_d(Wd, kb, d):
            """One [128, 128] weight block (rows kb, cols d) for the per-pair
            q/k projections — loaded one iteration ahead of use."""
            wds = sb.tile([P, P], f32, tag="wds", bufs=6, name="wds")
            nc.sync.dma_start(
                wds, Wd[kb * P : (kb + 1) * P, d * P : (d + 1) * P]
            )
            wd = sb.tile([P, P], bf16, tag="wd", bufs=40, name="wd")
            nc.vector.tensor_copy(wd, wds)
            return wd

        def load_transpose(Xd, slot0):
            """HBM row-major -> feature-major bf16 tiles xt[db] (128 x 1024).
            All three tensors share one 16-slot ring; V uses slots 0-7 first,
            Q takes 8-15, K reuses 0-7 once v_proj has consumed them.
            Transpose psum rotates across all three pools (6-deep ring)."""
            xt = [
                sb.tile([P, S], bf16, tag="xt", bufs=16, name=f"xt{slot0 + i}")
                for i in range(NB)
            ]
            for sblk in range(NB):
                xn = sb.tile([P, D], f32, tag="xn", bufs=2, name="xn")
                nc.sync.dma_start(xn, Xd[sblk * P : (sblk + 1) * P, :])
                xnb = sb.tile([P, D], bf16, tag="xnb", bufs=2, name="xnb")
                nc.vector.tensor_copy(xnb, xn)
                for db in range(NB):
                    r = (sblk * NB + db) % 3
                    if r == 0:
                        tp = big_tile("tp")[:, 0, :]
                    elif r == 1:
                        tp = half_tile("tph")
                    else:
                        tp = half_tile("tpv", tag="vph")
                    tpb = tp[:, 0:P].bitcast(bf16)[:, 0:P]
                    nc.tensor.transpose(
                        tpb, xnb[:, db * P : (db + 1) * P], identb
                    )
                    dst = xt[db][:, sblk * P : (sblk + 1) * P]
                    if (sblk + db) % 2 == 0:
                        nc.vector.tensor_copy(dst, tpb)
                    else:
                        nc.scalar.activation(dst, tpb, AF.Copy)
            return xt

        # ---- preps: V first (v_proj consumes vt before k_prep reuses the
        # ring slots — k_prep MUST be emitted after v_proj or the PE queue
        # deadlocks) -----------------------------------------------------
        with nc.named_scope("v_prep"):
            vt = load_transpose(Vd, 0)
        with nc.named_scope("q_prep"):
            qt = load_transpose(Qd, 8)

        # ---- V projection -> vaug (row-major, head-major, 65th=ones) ----
        with nc.named_scope("v_proj"):
            vaug = [
                sb.tile([P, H * 65], bf16, tag="vaug", bufs=NB, name=f"vaug{i}")
                for i in range(NB)
            ]
            for sblk in range(NB):
                nc.vector.memset(
                    vaug[sblk].rearrange("p (h c) -> p h c", c=65)[:, :, 64:65],
                    1.0,
                )
            wv = [[wload(WVd, kb, c) for c in range(2)] for kb in range(NB)]
            for sblk in range(NB):
                acc = big_tile("vacc")
                for kb in range(NB):
                    for c in range(2):
                        nc.tensor.matmul(
                            acc[:, c, :],
                            vt[kb][:, sblk * P : (sblk + 1) * P],
                            wv[kb][c],
                            start=(kb == 0),
                            stop=(kb == NB - 1),
                        )
                for c in range(2):
                    if with_bv:
                        nc.vector.tensor_add(
                            acc[:, c, :], acc[:, c, :], bvb[:, c * QC : (c + 1) * QC]
                        )
                    dst = vaug[sblk].rearrange("p (h c) -> p h c", c=65)[
                        :, c * 8 : (c + 1) * 8, 0:64
                    ]
                    nc.scalar.activation(
                        dst,
                        acc[:, c, :].rearrange("p (h c) -> p h c", c=64),
                        AF.Relu,
                    )

        # k_prep reuses V's xt ring slots — must come after v_proj
        with nc.named_scope("k_prep"):
            kt = load_transpose(Kd, 0)

        # ---- per-pair pipeline pieces -----------------------------------
        def gen_projd(xt, wstrips, d, bias_base, tag, out):
            """Projection output block relu(W[:, dblk].T @ X^T + b) as a
            generator yielding after each 4-matmul chunk (rider stream).
            Appends the evicted feature-major bf16 ring tile to `out`."""
            acc0 = half_tile("pacc0")
            acc1 = half_tile("pacc1")
            for g in range(4):
                for kb in (2 * g, 2 * g + 1):
                    wt = wstrips[kb]
                    first, last = kb == 0, kb == NB - 1
                    nc.tensor.matmul(
                        acc0, wt, xt[kb][:, 0:QC], start=first, stop=last
                    )
                    nc.tensor.matmul(
                        acc1, wt, xt[kb][:, QC:S], start=first, stop=last
                    )
                yield
            xpt = sb.tile([P, S], bf16, tag=tag, bufs=2, name=tag)
            for c, acc in ((0, acc0), (1, acc1)):
                nc.vector.tensor_scalar(
                    out=xpt[:, c * QC : (c + 1) * QC],
                    in0=acc,
                    scalar1=bqk[:, bias_base + d : bias_base + d + 1],
                    scalar2=0.0,
                    op0=ALU.add,
                    op1=ALU.max,
                )
            out.append(xpt)

        def emit_scores_unit(d, qpt, kpt, riders):
            """Scores + exp for head pair d (even head on PE rows 0-63, odd
            on 64-127, concurrent in the array). After each key block the
            rider generators (PV of pair d-1, projections for pair d+1) get
            one step each, so the PE stays busy while the scores matmuls
            throttle on the exp-gated psum ring."""
            ptA = sb.tile([P, NB, 2, QC], bf16, tag="pt", bufs=4, name="ptA")
            ptB = sb.tile([P, NB, 2, QC], bf16, tag="pt", bufs=4, name="ptB")
            for kb in range(NB):
                ksl = slice(kb * P, (kb + 1) * P)
                spA = big_tile("spA")
                spB = big_tile("spB")
                for qc in range(2):
                    qsl = slice(qc * QC, (qc + 1) * QC)
                    nc.tensor.matmul(
                        spA[:, qc, :], kpt[0:DK, ksl], qpt[0:DK, qsl],
                        start=True, stop=True,
                    )
                for qc in range(2):
                    qsl = slice(qc * QC, (qc + 1) * QC)
                    nc.tensor.matmul(
                        spB[:, qc, :], kpt[DK:P, ksl], qpt[DK:P, qsl],
                        start=True, stop=True,
                    )
                nc.scalar.activation(ptA[:, kb, :, :], spA, AF.Exp, scale=0.03125)
                nc.scalar.activation(ptB[:, kb, :, :], spB, AF.Exp, scale=0.03125)
                for rd in riders:
                    if rd is not None:
                        next(rd, None)
            return ptA, ptB

        ot = [
            sb.tile([P, S], bf16, tag="ot", bufs=NB, name=f"ot{i}")
            for i in range(NB)
        ]

        def emit_pv_tail(h, vp):
            """Softmax division: reciprocal of the denominator row on a
            DRAM-spread layout + DMA broadcast + multiply (all off PE/ACT)."""
            dbq, off = h // 2, (h % 2) * DK
            for qc in range(2):
                qsl = slice(qc * QC, (qc + 1) * QC)
                stage = sb.tile([65, QC], f32, tag="stage", bufs=3, name="stage")
                nc.vector.tensor_copy(stage, vp[qc][0:65, :])
                scr = dramp.tile([1, QC], f32, tag="scr", bufs=6, name="scr")
                nc.gpsimd.dma_start(scr, stage[64:65, :])
                rcp = sb.tile([DK, NB], f32, tag="rcp", bufs=3, name="rcp")
                nc.gpsimd.dma_start(
                    rcp, scr.rearrange("o (a b) -> a (o b)", a=DK)
                )
                nc.vector.reciprocal(rcp, rcp)
                scr2 = dramp.tile([1, QC], f32, tag="scr2", bufs=6, name="scr2")
                nc.gpsimd.dma_start(
                    scr2.rearrange("o (a b) -> a (o b)", a=DK), rcp
                )
                bc = sb.tile([DK, QC], f32, tag="bc", bufs=2, name="bc")
                nc.gpsimd.dma_start(bc, scr2.broadcast_to([DK, QC]))
                if off == 0:
                    nc.vector.tensor_mul(ot[dbq][0:DK, qsl], stage[0:DK, :], bc)
                else:
                    tmp = sb.tile([DK, QC], bf16, tag="tmp", bufs=2, name="tmp")
                    nc.vector.tensor_mul(tmp, stage[0:DK, :], bc)
                    nc.gpsimd.dma_start(ot[dbq][DK:P, qsl], tmp)

        def gen_pv_pair(d, ptA, ptB):
            """PV + division for head pair (2d, 2d+1), both q-chunks, yielded
            in 8 groups of 4 matmuls for interleaving with other PE work."""
            for hl, ptX in ((0, ptA), (1, ptB)):
                h = 2 * d + hl
                vp = [half_tile(f"vp{h}_{qc}", tag="vph") for qc in range(2)]
                for g in range(4):
                    for kb in (2 * g, 2 * g + 1):
                        for qc in range(2):
                            nc.tensor.matmul(
                                vp[qc][0:65, :],
                                vaug[kb][:, h * 65 : (h + 1) * 65],
                                ptX[:, kb, qc, :],
                                start=(kb == 0),
                                stop=(kb == NB - 1),
                            )
                    yield
                emit_pv_tail(h, vp)

        # ---- pipeline: preamble proj(0), then fused iterations ----------
        wnext = [
            [wload_d(WKd, kb, 0) for kb in range(NB)],
            [wload_d(WQd, kb, 0) for kb in range(NB)],
        ]
        with nc.named_scope("proj0"):
            cur = []
            for _ in itertools.chain(
                gen_projd(kt, wnext[0], 0, NB, "kpt", cur),
                gen_projd(qt, wnext[1], 0, 0, "qpt", cur),
            ):
                pass
            kpt_c, qpt_c = cur

        wo = []
        pend = None
        for d in range(NB):
            with nc.named_scope(f"it{d}"):
                if d + 1 < NB:
                    wnext = [
                        [wload_d(WKd, kb, d + 1) for kb in range(NB)],
                        [wload_d(WQd, kb, d + 1) for kb in range(NB)],
                    ]
                    nxt = []
                    g_proj = itertools.chain(
                        gen_projd(kt, wnext[0], d + 1, NB, "kpt", nxt),
                        gen_projd(qt, wnext[1], d + 1, 0, "qpt", nxt),
                    )
                else:
                    nxt, g_proj = None, None
                g_pv = gen_pv_pair(d - 1, *pend) if pend is not None else None
                if g_pv is not None:
                    # prime two PV groups: ready PE work while the first
                    # scores matmuls wait on the exp-gated psum ring slot
                    next(g_pv, None)
                    next(g_pv, None)
                pend = emit_scores_unit(d, qpt_c, kpt_c, [g_pv, g_proj])
                for g in (g_pv, g_proj):
                    if g is not None:
                        for _ in g:
                            pass
                if nxt is not None:
                    kpt_c, qpt_c = nxt
                if 4 <= d < 7:
                    # spread the o_proj weight loads across iterations 4-6
                    wo.append([wload(WOd, 2 * (d - 4), c) for c in range(2)])
                    wo.append(
                        [wload(WOd, 2 * (d - 4) + 1, c) for c in range(2)]
                    )
                    if d == 6:
                        wo.append([wload(WOd, 6, c) for c in range(2)])
                        wo.append([wload(WOd, 7, c) for c in range(2)])

        # ---- tail: o_proj chains (db 0-6 open early; db7 lands last) ----
        def oproj_open(sblk, accs, rider=None):
            for db in range(NB - 1):
                for c in range(2):
                    nc.tensor.matmul(
                        accs[c],
                        ot[db][:, sblk * P : (sblk + 1) * P],
                        wo[db][c],
                        start=(db == 0),
                        stop=False,
                    )
                if rider is not None and db % 2 == 1:
                    next(rider, None)
            return accs

        def oproj_close(sblk, accs):
            for c in range(2):
                nc.tensor.matmul(
                    accs[c],
                    ot[NB - 1][:, sblk * P : (sblk + 1) * P],
                    wo[NB - 1][c],
                    start=False,
                    stop=True,
                )
            for c in range(2):
                if with_bo:
                    nc.vector.tensor_add(
                        accs[c], accs[c], bob[:, c * QC : (c + 1) * QC]
                    )
                # 4-deep obuf ring + SWDGE queue (idle at the tail, unlike
                # Sync): keeps the relu evictions from serializing on the
                # out-DMA completions
                o = sb.tile([P, QC], f32, tag="obuf", bufs=4, name="obuf")
                nc.scalar.activation(o, accs[c], AF.Relu)
                nc.gpsimd.dma_start(
                    outd[sblk * P : (sblk + 1) * P, c * QC : (c + 1) * QC], o
                )

        def chain_accs(sblk):
            """Chain accumulators: sblk 0,1,4,5 use the two big-pool tiles;
            2,3,6,7 pair a half tile with a vph tile (both free at the tail)."""
            if sblk % 4 < 2:
                t = big_tile(f"oacc{sblk}")
                return [t[:, 0, :], t[:, 1, :]]
            return [half_tile(f"oh{sblk}"), half_tile(f"ov{sblk}", tag="vph")]

        with nc.named_scope("o_proj"):
            gpv = gen_pv_pair(NB - 1, *pend)
            next(gpv, None)
            chains = {}
            # big-pool chains ride the last PV pair; the split (half+vph)
            # chains may only allocate after gpv fully drains, else their
            # matmuls wait on vph stage-copies that sit later in the PE FIFO
            for sblk in range(2):
                chains[sblk] = oproj_open(sblk, chain_accs(sblk), rider=gpv)
            for _ in gpv:
                pass
            for sblk in range(2, 4):
                chains[sblk] = oproj_open(sblk, chain_accs(sblk))
            for sblk in range(4, NB):
                oproj_close(sblk - 4, chains.pop(sblk - 4))
                chains[sblk] = oproj_open(sblk, chain_accs(sblk))
            for sblk in range(NB - 4, NB):
                oproj_close(sblk, chains.pop(sblk))

    _split_wide_waits(nc)
    return nc


_NC_CACHE = {}


def kernel(Q, K, V, WQ, bQ, WK, bK, WV, bV, WO, bO, h):
    Q, K, V = (np.ascontiguousarray(np.asarray(x, np.float32)) for x in (Q, K, V))
    WQ, WK, WV, WO = (
        np.ascontiguousarray(np.asarray(x, np.float32)) for x in (WQ, WK, WV, WO)
    )
    bQ, bK, bV, bO = (
        np.ascontiguousarray(np.asarray(x, np.float32)) for x in (bQ, bK, bV, bO)
    )
    h = int(np.asarray(h))
    assert h == H, f"kernel specialized for h=16, got {h}"
    B = Q.shape[0]
    assert Q.shape == (B, S, D) and B == N_CORES

    key = (bool(np.any(bV)), bool(np.any(bO)))
    if key not in _NC_CACHE:
        _NC_CACHE[key] = _build_nc(*key)
    nc = _NC_CACHE[key]

    in_maps = [
        {
            "Q": Q[b], "K": K[b], "V": V[b],
            "WQ": WQ, "WK": WK, "WV": WV, "WO": WO,
            "bQ": bQ, "bK": bK, "bV": bV, "bO": bO,
        }
        for b in range(B)
    ]
    trace = os.environ.get("BASS_MHA_TRACE") == "1"
    res = run_bass_kernel_spmd(
        nc, in_maps, core_ids=list(range(N_CORES)), trace=trace
    )
    if trace:
        kernel.last_results = res
    return np.stack([res.results[b]["out"] for b in range(B)], axis=0)
